# revision 1
# baseline (speedup 1.0000x reference)
"""Trainium2 Bass kernel for nn_EnhancedEdgeScorer (gnn_message_passing).

Sharding: data-parallel over nodes (2048/core) and edges (8192/core) on 8
NeuronCores.  Per layer, each core computes K/V for its node shard, the
shards are AllGathered, and each core gathers its nodes' neighbor K/V rows
with dma_gather.  Key algebraic folds:
  - k/v are projected BEFORE the neighbor gather (gather commutes with the
    row-linear projection), turning the reference's (N*M,H)@(H,H) matmuls
    into (N,H)@(H,H).
  - k-bias drops out (softmax shift invariance); v-bias folds into the
    out-projection bias; the 1/sqrt(dh) scale folds into wq/bq.
Everything dense runs on the PE in bf16 with fp32 PSUM accumulation.
"""

import numpy as np
import ml_dtypes
from contextlib import ExitStack

import concourse.bass as bass
from concourse import bacc
import concourse.tile as tile
import concourse.mybir as mybir
from concourse.masks import make_identity
from concourse.bass_utils import run_bass_kernel_spmd

BF16 = mybir.dt.bfloat16
F32 = mybir.dt.float32
I16 = mybir.dt.int16

N, M, H, HEADS, L, E = 16384, 32, 256, 4, 3, 65536
DH = H // HEADS
T, V, CD = 8, 17, 64
TOTAL = H // 2 + 2 * CD + H // 4  # 320
NC = 8
NL = N // NC      # 2048 nodes per core
EL = E // NC      # 8192 edges per core
P = 128
NT = NL // P      # 16 node tiles per core
ET = EL // 512    # 16 edge chunks per core
NEG = -30.0       # additive pad-mask value (exp(-30) ~ 1e-13)

_bf = lambda a: np.ascontiguousarray(a.astype(ml_dtypes.bfloat16))
_f32 = lambda a: np.ascontiguousarray(a.astype(np.float32))


def _wrap16(idx):
    """Flat index list -> [128, len/16] int16 layout dma_gather expects
    (the 16-partition block is replicated for each of the 8 Q7 cores)."""
    idx = np.asarray(idx, dtype=np.int16)
    assert idx.size % 16 == 0
    return np.ascontiguousarray(np.tile(idx.reshape(-1, 16).T, (8, 1)))


# --------------------------------------------------------------------------
# Bass program (SPMD; per-core differences enter only through input data)
# --------------------------------------------------------------------------

def build_program():
    nc = bacc.Bacc(num_devices=NC)

    dp = lambda nm, shp, dt: nc.declare_dram_parameter(nm, list(shp), dt, isOutput=False)

    # ---- weights (same on all cores) ----
    type_tab = dp("type_tab", [T, H // 2], BF16)          # gather-T, elem 128
    cat_tab = dp("cat_tab", [V * V, 2 * CD], BF16)        # combined cat embeds
    dw = dp("dw", [1, H // 4], F32)                       # degree_w row
    db = dp("db", [H // 4], F32)
    projWT = dp("projWT", [3, P, H], BF16)                # proj_w.T in 3 row-chunks (zero padded)
    proj_b = dp("proj_b", [H], F32)
    wqT = dp("wqT", [L, 2, P, H], BF16)                   # (wq*scale).T row-chunks
    bq = dp("bq", [L, H], F32)                            # bq*scale
    wkT = dp("wkT", [L, 2, P, H], BF16)
    wvT = dp("wvT", [L, 2, P, H], BF16)
    woT = dp("woT", [L, 2, P, H], BF16)
    bo = dp("bo", [L, H], F32)                            # out_b + out_w@bv
    w1T = dp("w1T", [4, P, H], BF16)                  # mlp_w1.T eu/ev row-chunks
    w1eT = dp("w1eT", [2, H], BF16)                   # mlp_w1.T edge-feat rows
    b1 = dp("b1", [P, 2], F32)                            # b1 as [128, chunk]
    w2T = dp("w2T", [2, P, H // 2], BF16)
    b2 = dp("b2", [H // 2], F32)
    w3T = dp("w3T", [P, 1], BF16)
    b3 = dp("b3", [1], F32)

    # ---- per-core data ----
    idx_kv = dp("idx_kv", [P, NT * (P * M // 16)], I16)  # m-major ctx idx per node tile
    idx_type = dp("idx_type", [P, NL // 16], I16)
    idx_cat = dp("idx_cat", [P, NL // 16], I16)
    idx_u = dp("idx_u", [P, EL // 16], I16)
    idx_v = dp("idx_v", [P, EL // 16], I16)
    logd = dp("logd", [1, NL], F32)
    kp = dp("kp", [NL, M], F32)                           # additive pad mask (0 / NEG)
    efT = dp("efT", [2, EL], BF16)

    out_d = nc.declare_dram_parameter("out", [EL], F32, isOutput=True)

    # ---- internal DRAM ----
    kloc = nc.dram_tensor("kloc", [NL, H], BF16)
    vloc = nc.dram_tensor("vloc", [NL, H], BF16)
    xloc = nc.dram_tensor("xloc", [NL, H], BF16)
    kall = nc.dram_tensor("kall", [N, H], BF16, addr_space="Shared")
    vall = nc.dram_tensor("vall", [N, H], BF16, addr_space="Shared")
    xall = nc.dram_tensor("xall", [N, H], BF16, addr_space="Shared")

    groups = [list(range(NC))]
    Alu = mybir.AluOpType
    Act = mybir.ActivationFunctionType

    with tile.TileContext(nc) as tc, ExitStack() as ctx:
        const = ctx.enter_context(tc.tile_pool(name="const", bufs=1))
        xpool = ctx.enter_context(tc.tile_pool(name="xpool", bufs=1))

        # ---------------- constants into SBUF ----------------
        gather = nc.gpsimd.dma_gather
        reg_nl = nc.gpsimd.to_reg(NL)
        reg_pm = nc.gpsimd.to_reg(P * M)
        reg_e2 = nc.gpsimd.to_reg(EL // 2)

        ident = const.tile([P, P], BF16)
        make_identity(nc, ident)

        def bcast_row(dram_ap, n, name):
            t = const.tile([P, n], F32, tag=name, name=name)
            src = bass.AP(tensor=dram_ap.tensor, offset=dram_ap.offset,
                          ap=[[0, P]] + dram_ap.ap)
            nc.sync.dma_start(out=t[:], in_=src)
            return t

        pb_b = bcast_row(proj_b[:], H, "pb")
        bq_b = [bcast_row(bq[ll, :], H, f"bq{ll}") for ll in range(L)]
        bo_b = [bcast_row(bo[ll, :], H, f"bo{ll}") for ll in range(L)]

        db_sb = const.tile([H // 4, 1], F32)
        nc.sync.dma_start(out=db_sb[:], in_=db.rearrange("(p o) -> p o", o=1))
        dw_sb = const.tile([1, H // 4], F32)
        nc.sync.dma_start(out=dw_sb[:], in_=dw[:])
        b1_sb = const.tile([P, 2], F32)
        nc.sync.dma_start(out=b1_sb[:], in_=b1[:])
        b2_sb = const.tile([H // 2, 1], F32)
        nc.sync.dma_start(out=b2_sb[:], in_=b2.rearrange("(p o) -> p o", o=1))
        b3_sb = const.tile([1, 1], F32)
        nc.sync.dma_start(out=b3_sb[:], in_=b3.rearrange("(p o) -> p o", o=1))

        ikv_sb = const.tile([P, NT * P * M // 16], I16)
        nc.sync.dma_start(out=ikv_sb[:], in_=idx_kv[:])
        ity_sb = const.tile([P, NL // 16], I16)
        nc.sync.dma_start(out=ity_sb[:], in_=idx_type[:])
        ica_sb = const.tile([P, NL // 16], I16)
        nc.sync.dma_start(out=ica_sb[:], in_=idx_cat[:])
        iu_sb = const.tile([P, EL // 16], I16)
        nc.sync.dma_start(out=iu_sb[:], in_=idx_u[:])
        iv_sb = const.tile([P, EL // 16], I16)
        nc.sync.dma_start(out=iv_sb[:], in_=idx_v[:])

        kp_sb = const.tile([P, NT, M], F32)
        nc.sync.dma_start(out=kp_sb[:], in_=kp.rearrange("(t p) m -> p t m", p=P))
        logd_sb = const.tile([1, NL], F32)
        nc.sync.dma_start(out=logd_sb[:], in_=logd[:])

        pw_sb = const.tile([P, 3, H], BF16)
        nc.sync.dma_start(out=pw_sb[:], in_=projWT.rearrange("c p o -> p c o"))
        w1_sb = const.tile([P, 4, H], BF16)
        nc.sync.dma_start(out=w1_sb[:], in_=w1T.rearrange("c p o -> p c o"))
        w1e_sb = const.tile([2, H], BF16)
        nc.sync.dma_start(out=w1e_sb[:], in_=w1eT[:])
        w2_sb = const.tile([P, 2, H // 2], BF16)
        nc.sync.dma_start(out=w2_sb[:], in_=w2T.rearrange("c p o -> p c o"))
        w3_sb = const.tile([P, 1], BF16)
        nc.sync.dma_start(out=w3_sb[:], in_=w3T[:])

        x_sb = xpool.tile([P, NT, H], BF16)

        # ---------------- node feature encoding (scoped pools) ----------------
        with ExitStack() as ectx:
            enc = ectx.enter_context(tc.tile_pool(name="enc", bufs=1))
            epsum = ectx.enter_context(tc.tile_pool(name="epsum", bufs=2, space="PSUM"))
            teT = enc.tile([P, NL], BF16)
            gather(teT.rearrange("p (c n) -> p c n", c=1), type_tab[:],
                                 ity_sb[:], NL, reg_nl, H // 2, transpose=True, single_packet=False)
            ccT = enc.tile([P, NL], BF16)
            gather(ccT.rearrange("p (c n) -> p c n", c=1), cat_tab[:],
                                 ica_sb[:], NL, reg_nl, 2 * CD, transpose=True, single_packet=False)
            deT = enc.tile([P, NL], BF16)
            nc.vector.memset(deT[:], 0.0)
            for s in range(NL // 512):
                pd = epsum.tile([H // 4, 512], F32, tag="pdeg", name="pd")
                nc.tensor.matmul(pd[:], dw_sb[:], logd_sb[:, s * 512:(s + 1) * 512],
                                 start=True, stop=True)
                nc.scalar.activation(deT[0:H // 4, s * 512:(s + 1) * 512], pd[:],
                                     Act.Relu, bias=db_sb[:])
            for g in range(NT):
                px = epsum.tile([P, H], F32, tag="px", name="px")
                cs = slice(g * P, (g + 1) * P)
                nc.tensor.matmul(px[:], teT[:, cs], pw_sb[:, 0, :], start=True, stop=False)
                nc.tensor.matmul(px[:], ccT[:, cs], pw_sb[:, 1, :], start=False, stop=False)
                nc.tensor.matmul(px[:], deT[:, cs], pw_sb[:, 2, :], start=False, stop=True)
                nc.vector.tensor_tensor(x_sb[:, g, :], px[:], pb_b[:], op=Alu.add)

        work = ctx.enter_context(tc.tile_pool(name="work", bufs=1))
        gath = ctx.enter_context(tc.tile_pool(name="gath", bufs=2))
        att = ctx.enter_context(tc.tile_pool(name="att", bufs=2))
        psum = ctx.enter_context(tc.tile_pool(name="psum", bufs=2, space="PSUM"))
        psum1 = ctx.enter_context(tc.tile_pool(name="psum1", bufs=2, space="PSUM"))

        # ---------------- attention layers ----------------
        for ll in range(L):
            wq_sb = work.tile([P, 2, H], BF16, tag="wq", name="wq")
            wk_sb = work.tile([P, 2, H], BF16, tag="wk", name="wk")
            wv_sb = work.tile([P, 2, H], BF16, tag="wv", name="wv")
            wo_sb = work.tile([P, 2, H], BF16, tag="wo", name="wo")
            nc.sync.dma_start(out=wq_sb[:], in_=wqT[ll].rearrange("c p o -> p c o"))
            nc.sync.dma_start(out=wk_sb[:], in_=wkT[ll].rearrange("c p o -> p c o"))
            nc.sync.dma_start(out=wv_sb[:], in_=wvT[ll].rearrange("c p o -> p c o"))
            nc.sync.dma_start(out=wo_sb[:], in_=woT[ll].rearrange("c p o -> p c o"))

            # x^T tiles (lhsT for projections)
            xT = work.tile([P, 2, NT, P], BF16, tag="xT", name="xT")
            for g in range(NT):
                for c in range(2):
                    pt = psum1.tile([P, P], BF16, tag="ptr", name="pt")
                    nc.tensor.transpose(pt[:], x_sb[:, g, c * P:(c + 1) * P], ident[:])
                    nc.scalar.activation(xT[:, c, g, :], pt[:], Act.Copy)

            q_sb = work.tile([P, NT, H], BF16, tag="q", name="q_sb")
            kall_pview = kloc.rearrange("(t p) o -> p t o", p=P)
            vall_pview = vloc.rearrange("(t p) o -> p t o", p=P)
            for g in range(NT):
                pq = psum.tile([P, H], F32, tag="pmm", name="pq")
                nc.tensor.matmul(pq[:], xT[:, 0, g, :], wq_sb[:, 0, :], start=True, stop=False)
                nc.tensor.matmul(pq[:], xT[:, 1, g, :], wq_sb[:, 1, :], start=False, stop=True)
                nc.vector.tensor_tensor(q_sb[:, g, :], pq[:], bq_b[ll][:], op=Alu.add)
                pk = psum.tile([P, H], F32, tag="pmm", name="pk")
                nc.tensor.matmul(pk[:], xT[:, 0, g, :], wk_sb[:, 0, :], start=True, stop=False)
                nc.tensor.matmul(pk[:], xT[:, 1, g, :], wk_sb[:, 1, :], start=False, stop=True)
                kev = work.tile([P, H], BF16, tag="kev", name="kev", bufs=2)
                nc.scalar.activation(kev[:], pk[:], Act.Copy)
                nc.sync.dma_start(out=kall_pview[:, g, :], in_=kev[:])
                pv = psum.tile([P, H], F32, tag="pmm", name="pv")
                nc.tensor.matmul(pv[:], xT[:, 0, g, :], wv_sb[:, 0, :], start=True, stop=False)
                nc.tensor.matmul(pv[:], xT[:, 1, g, :], wv_sb[:, 1, :], start=False, stop=True)
                vev = work.tile([P, H], BF16, tag="vev", name="vev", bufs=2)
                nc.scalar.activation(vev[:], pv[:], Act.Copy)
                nc.sync.dma_start(out=vall_pview[:, g, :], in_=vev[:])

            nc.gpsimd.collective_compute("AllGather", Alu.bypass, replica_groups=groups,
                                         ins=[kloc[:]], outs=[kall[:]])
            nc.gpsimd.collective_compute("AllGather", Alu.bypass, replica_groups=groups,
                                         ins=[vloc[:]], outs=[vall[:]])

            for t in range(NT):
                isl = ikv_sb[:, t * (P * M // 16):(t + 1) * (P * M // 16)]
                kg = gath.tile([P, M, H], BF16, tag="kg", name="kg")
                gather(kg[:], kall[:], isl, P * M, reg_pm, H, single_packet=False)
                vg = gath.tile([P, M, H], BF16, tag="vg", name="vg")
                gather(vg[:], vall[:], isl, P * M, reg_pm, H, single_packet=False)

                # scores: s[n,m,h] = sum_d q*k  (d-tree reduce, ping-pong pp<->ta)
                pp = att.tile([P, M, HEADS, DH], BF16, tag="pp", name="pp")
                qb = q_sb[:, t, None, :].to_broadcast([P, M, H])
                nc.vector.tensor_tensor(pp.rearrange("p m h d -> p m (h d)"),
                                        kg.rearrange("p m o -> p m o"), qb, op=Alu.mult)
                ta = att.tile([P, M, HEADS, DH // 2], BF16, tag="ta", name="ta")
                nc.vector.tensor_tensor(ta[:], pp[:, :, :, 0:32], pp[:, :, :, 32:64], op=Alu.add)
                nc.vector.tensor_tensor(pp[:, :, :, 0:16], ta[:, :, :, 0:16], ta[:, :, :, 16:32], op=Alu.add)
                nc.vector.tensor_tensor(ta[:, :, :, 0:8], pp[:, :, :, 0:8], pp[:, :, :, 8:16], op=Alu.add)
                nc.vector.tensor_tensor(pp[:, :, :, 0:4], ta[:, :, :, 0:4], ta[:, :, :, 4:8], op=Alu.add)
                nc.vector.tensor_tensor(ta[:, :, :, 0:2], pp[:, :, :, 0:2], pp[:, :, :, 2:4], op=Alu.add)
                s_m = att.tile([P, M, HEADS], F32, tag="sm", name="s_m")
                nc.vector.tensor_tensor(s_m[:], ta[:, :, :, 0], ta[:, :, :, 1], op=Alu.add)

                kpb = kp_sb[:, t, :, None].to_broadcast([P, M, HEADS])
                nc.vector.tensor_tensor(s_m[:], s_m[:], kpb, op=Alu.add)
                es = att.tile([P, M, HEADS], F32, tag="es", name="es")
                nc.scalar.activation(es[:], s_m[:], Act.Exp)
                sums = att.tile([P, HEADS], F32, tag="sums", name="sums")
                nc.vector.tensor_reduce(sums[:], es.rearrange("p m h -> p h m"),
                                        axis=mybir.AxisListType.X, op=Alu.add)
                rs = att.tile([P, HEADS], F32, tag="rs", name="rs")
                nc.vector.reciprocal(rs[:], sums[:])
                attw = att.tile([P, M, HEADS], BF16, tag="attw", name="attw")
                nc.vector.tensor_tensor(attw[:], es[:],
                                        rs[:, None, :].to_broadcast([P, M, HEADS]), op=Alu.mult)

                # AV: o[n,:] = sum_m attw * v  (m-tree, ping-pong av<->tm)
                av = att.tile([P, M, H], BF16, tag="pp", name="av")
                nc.vector.tensor_tensor(av.rearrange("p m (h d) -> p m h d", h=HEADS),
                                        vg.rearrange("p m (h d) -> p m h d", h=HEADS),
                                        attw[:, :, :, None].to_broadcast([P, M, HEADS, DH]),
                                        op=Alu.mult)
                tm = att.tile([P, M // 2, H], BF16, tag="ta", name="tm")
                nc.vector.tensor_tensor(tm[:], av[:, 0:16, :], av[:, 16:32, :], op=Alu.add)
                nc.vector.tensor_tensor(av[:, 0:8, :], tm[:, 0:8, :], tm[:, 8:16, :], op=Alu.add)
                nc.vector.tensor_tensor(tm[:, 0:4, :], av[:, 0:4, :], av[:, 4:8, :], op=Alu.add)
                nc.vector.tensor_tensor(av[:, 0:2, :], tm[:, 0:2, :], tm[:, 2:4, :], op=Alu.add)
                o_sb = att.tile([P, H], BF16, tag="o", name="o_sb")
                nc.vector.tensor_tensor(o_sb[:], av[:, 0, :], av[:, 1, :], op=Alu.add)

                # out-proj + relu -> x
                oT = att.tile([P, 2, P], BF16, tag="oT", name="oT")
                for c in range(2):
                    pt = psum1.tile([P, P], BF16, tag="ptr", name="pt")
                    nc.tensor.transpose(pt[:], o_sb[:, c * P:(c + 1) * P], ident[:])
                    nc.scalar.activation(oT[:, c, :], pt[:], Act.Copy)
                pxn = psum.tile([P, H], F32, tag="pmm", name="pxn")
                nc.tensor.matmul(pxn[:], oT[:, 0, :], wo_sb[:, 0, :], start=True, stop=False)
                nc.tensor.matmul(pxn[:], oT[:, 1, :], wo_sb[:, 1, :], start=False, stop=True)
                nc.vector.tensor_tensor(x_sb[:, t, :], pxn[:], bo_b[ll][:], op=Alu.add)
                nc.vector.tensor_scalar_max(x_sb[:, t, :], x_sb[:, t, :], 0.0)

        # ---------------- edge MLP ----------------
        nc.sync.dma_start(out=xloc.rearrange("(t p) o -> p t o", p=P), in_=x_sb[:])
        nc.gpsimd.collective_compute("AllGather", Alu.bypass, replica_groups=groups,
                                     ins=[xloc[:]], outs=[xall[:]])

        EH = EL // 2
        for half in range(2):
            hsl = slice(half * (EH // 16), (half + 1) * (EH // 16))
            ug = gath.tile([P, 2, EH], BF16, tag="kg", name="ug")
            gather(ug[:], xall[:], iu_sb[:, hsl], EH, reg_e2, H,
                                 transpose=True, single_packet=False)
            vg2 = gath.tile([P, 2, EH], BF16, tag="vg", name="vg2")
            gather(vg2[:], xall[:], iv_sb[:, hsl], EH, reg_e2, H,
                                 transpose=True, single_packet=False)
            for e in range(EH // 512):
                eg = half * (EH // 512) + e
                esl = slice(e * 512, (e + 1) * 512)
                ef_sb = att.tile([2, 512], BF16, tag="ef", name="ef_sb")
                nc.sync.dma_start(out=ef_sb[:], in_=efT[:, eg * 512:(eg + 1) * 512])
                h1T = att.tile([P, 2, 512], BF16, tag="h1T", name="h1T")
                for oc in range(2):
                    ph = psum.tile([P, 512], F32, tag="pbig", name="ph")
                    ocs = slice(oc * P, (oc + 1) * P)
                    nc.tensor.matmul(ph[:], w1_sb[:, 0, ocs], ug[:, 0, esl], start=True, stop=False)
                    nc.tensor.matmul(ph[:], w1_sb[:, 1, ocs], ug[:, 1, esl], start=False, stop=False)
                    nc.tensor.matmul(ph[:], w1_sb[:, 2, ocs], vg2[:, 0, esl], start=False, stop=False)
                    nc.tensor.matmul(ph[:], w1_sb[:, 3, ocs], vg2[:, 1, esl], start=False, stop=False)
                    nc.tensor.matmul(ph[:], w1e_sb[:, ocs], ef_sb[:], start=False, stop=True)
                    nc.scalar.activation(h1T[:, oc, :], ph[:], Act.Relu, bias=b1_sb[:, oc:oc + 1])
                ph2 = psum.tile([P, 512], F32, tag="pbig", name="ph2")
                nc.tensor.matmul(ph2[0:H // 2, :], w2_sb[:, 0, :], h1T[:, 0, :], start=True, stop=False)
                nc.tensor.matmul(ph2[0:H // 2, :], w2_sb[:, 1, :], h1T[:, 1, :], start=False, stop=True)
                h2T = att.tile([H // 2, 512], BF16, tag="h2T", name="h2T")
                nc.scalar.activation(h2T[:], ph2[0:H // 2, :], Act.Relu, bias=b2_sb[:])
                pl = psum1.tile([1, 512], F32, tag="pl", name="pl")
                nc.tensor.matmul(pl[:], w3_sb[:, :], h2T[:], start=True, stop=True)
                lo = att.tile([1, 512], F32, tag="lo", name="lo")
                nc.scalar.activation(lo[:], pl[:], Act.Identity, bias=b3_sb[:])
                nc.sync.dma_start(out=out_d.rearrange("(a b) -> a b", a=ET)[eg, None, :], in_=lo[:])

    nc.finalize()
    return nc


# --------------------------------------------------------------------------
# Host-side prep + runner
# --------------------------------------------------------------------------

_CACHE = {}


def _prep_maps(inputs):
    f = {k: np.asarray(v) for k, v in inputs.items()}
    scale = 1.0 / np.sqrt(np.float32(DH))

    cat0, cat1 = f["cat_embed0"].astype(np.float32), f["cat_embed1"].astype(np.float32)
    cat_tab = np.zeros((V * V, 2 * CD), np.float32)
    for i0 in range(V):
        for i1 in range(V):
            cat_tab[i0 * V + i1] = np.concatenate([cat0[i0], cat1[i1]])

    projWT = np.zeros((3, P, H), np.float32)
    pwt = f["proj_w"].astype(np.float32).T  # [320, 256]
    projWT[0] = pwt[0:128]
    projWT[1] = pwt[128:256]
    projWT[2, 0:64] = pwt[256:320]

    wqT = np.empty((L, 2, P, H), np.float32)
    wkT = np.empty((L, 2, P, H), np.float32)
    wvT = np.empty((L, 2, P, H), np.float32)
    woT = np.empty((L, 2, P, H), np.float32)
    bq = np.empty((L, H), np.float32)
    bo = np.empty((L, H), np.float32)
    for ll in range(L):
        w = f["in_proj_w"][ll].astype(np.float32)
        b = f["in_proj_b"][ll].astype(np.float32)
        wq, wk, wv = w[0:H], w[H:2 * H], w[2 * H:3 * H]
        bq[ll] = b[0:H] * scale
        bv = b[2 * H:3 * H]
        for c in range(2):
            wqT[ll, c] = (wq * scale).T[c * P:(c + 1) * P]
            wkT[ll, c] = wk.T[c * P:(c + 1) * P]
            wvT[ll, c] = wv.T[c * P:(c + 1) * P]
            woT[ll, c] = f["out_w"][ll].astype(np.float32).T[c * P:(c + 1) * P]
        bo[ll] = f["out_b"][ll].astype(np.float32) + f["out_w"][ll].astype(np.float32) @ bv

    w1 = f["mlp_w1"].astype(np.float32)      # [256, 514]
    w1T_full = w1.T                           # [514, 256]
    w1T = np.stack([w1T_full[c * P:(c + 1) * P] for c in range(4)])
    w1eT = w1T_full[512:514]
    b1 = f["mlp_b1"].astype(np.float32).reshape(2, P).T  # [128, 2]
    w2T = np.stack([f["mlp_w2"].astype(np.float32).T[c * P:(c + 1) * P] for c in range(2)])
    w3T = f["mlp_w3"].astype(np.float32).T   # [128, 1]

    shared = {
        "type_tab": _bf(f["type_embed"]),
        "cat_tab": _bf(cat_tab),
        "dw": _f32(f["degree_w"].reshape(1, -1)),
        "db": _f32(f["degree_b"]),
        "projWT": _bf(projWT),
        "proj_b": _f32(f["proj_b"]),
        "wqT": _bf(wqT), "bq": _f32(bq),
        "wkT": _bf(wkT), "wvT": _bf(wvT),
        "woT": _bf(woT), "bo": _f32(bo),
        "w1T": _bf(w1T), "w1eT": _bf(w1eT), "b1": _f32(b1),
        "w2T": _bf(w2T), "b2": _f32(f["mlp_b2"]),
        "w3T": _bf(w3T), "b3": _f32(f["mlp_b3"]),
    }

    ctx = f["context_indices"].astype(np.int64)
    kpm = f["key_padding_mask"].astype(bool)
    maps = []
    for c in range(NC):
        ns = slice(c * NL, (c + 1) * NL)
        es = slice(c * EL, (c + 1) * EL)
        ctx_c = ctx[ns]  # [2048, 32]
        idx_kv = np.concatenate(
            [ctx_c[t * P:(t + 1) * P].T.flatten() for t in range(NT)])
        m = dict(shared)
        m["idx_kv"] = _wrap16(idx_kv)
        m["idx_type"] = _wrap16(f["type_idx"][ns])
        m["idx_cat"] = _wrap16(f["cat_idx"][ns, 0] * V + f["cat_idx"][ns, 1])
        m["idx_u"] = _wrap16(f["u_idx"][es])
        m["idx_v"] = _wrap16(f["v_idx"][es])
        m["logd"] = _f32(f["log_degree"][ns].reshape(1, NL))
        m["kp"] = _f32(np.where(kpm[ns], NEG, 0.0))
        m["efT"] = _bf(f["edge_feats"][es].T)
        maps.append(m)
    return maps


def kernel(**inputs):
    if "nc" not in _CACHE:
        _CACHE["nc"] = build_program()
    nc = _CACHE["nc"]
    maps = _prep_maps(inputs)
    res = run_bass_kernel_spmd(nc, maps, core_ids=list(range(NC)))
    return np.concatenate([res.results[c]["out"] for c in range(NC)]).astype(np.float32)


if __name__ == "__main__":
    nc = build_program()
    print("program built OK")



# revision 11
# speedup vs baseline: 1.5011x; 1.5011x over previous
"""Trainium2 Bass kernel for nn_EnhancedEdgeScorer (gnn_message_passing).

Sharding: data-parallel over nodes (2048/core) and edges (8192/core) on 8
NeuronCores.  Per layer, each core computes K/V for its node shard, the
shards are AllGathered, and each core gathers its nodes' neighbor K/V rows
with dma_gather.  Key algebraic folds:
  - k/v are projected BEFORE the neighbor gather (gather commutes with the
    row-linear projection), turning the reference's (N*M,H)@(H,H) matmuls
    into (N,H)@(H,H).
  - k-bias drops out (softmax shift invariance); v-bias folds into the
    out-projection bias; the 1/sqrt(dh) scale folds into wq/bq.
Everything dense runs on the PE in bf16 with fp32 PSUM accumulation.
"""

import numpy as np
import ml_dtypes
from contextlib import ExitStack

import concourse.bass as bass
from concourse import bacc
import concourse.tile as tile
import concourse.mybir as mybir
from concourse.masks import make_identity
from concourse.bass_utils import run_bass_kernel_spmd

BF16 = mybir.dt.bfloat16
F32 = mybir.dt.float32
I16 = mybir.dt.int16

N, M, H, HEADS, L, E = 16384, 32, 256, 4, 3, 65536
DH = H // HEADS
T, V, CD = 8, 17, 64
TOTAL = H // 2 + 2 * CD + H // 4  # 320
NC = 8
NL = N // NC      # 2048 nodes per core
EL = E // NC      # 8192 edges per core
P = 128
NT = NL // P      # 16 node tiles per core
ET = EL // 512    # 16 edge chunks per core
NEG = -30.0       # additive pad-mask value (exp(-30) ~ 1e-13)

_bf = lambda a: np.ascontiguousarray(a.astype(ml_dtypes.bfloat16))
_f32 = lambda a: np.ascontiguousarray(a.astype(np.float32))


def _wrap16(idx):
    """Flat index list -> [128, len/16] int16 layout dma_gather expects
    (the 16-partition block is replicated for each of the 8 Q7 cores)."""
    idx = np.asarray(idx, dtype=np.int16)
    assert idx.size % 16 == 0
    return np.ascontiguousarray(np.tile(idx.reshape(-1, 16).T, (8, 1)))


# --------------------------------------------------------------------------
# Bass program (SPMD; per-core differences enter only through input data)
# --------------------------------------------------------------------------

def build_program():
    nc = bacc.Bacc(num_devices=NC)

    dp = lambda nm, shp, dt: nc.declare_dram_parameter(nm, list(shp), dt, isOutput=False)

    # ---- weights (same on all cores) ----
    type_tab = dp("type_tab", [T, H // 2], BF16)          # gather-T, elem 128
    cat_tab = dp("cat_tab", [V * V, 2 * CD], BF16)        # combined cat embeds
    dw = dp("dw", [1, H // 4], F32)                       # degree_w row
    db = dp("db", [H // 4], F32)
    projWT = dp("projWT", [3, P, H], BF16)                # proj_w.T in 3 row-chunks (zero padded)
    proj_b = dp("proj_b", [H], F32)
    wqT = dp("wqT", [L, 2, P, H], BF16)                   # (wq*scale).T row-chunks
    bq = dp("bq", [L, H], F32)                            # bq*scale
    wkT = dp("wkT", [L, 2, P, H], BF16)
    wvT = dp("wvT", [L, 2, P, H], BF16)
    woT = dp("woT", [L, 2, P, H], BF16)
    bo = dp("bo", [L, H], F32)                            # out_b + out_w@bv
    w1T = dp("w1T", [4, P, H], BF16)                  # mlp_w1.T eu/ev row-chunks
    w1eT = dp("w1eT", [2, H], BF16)                   # mlp_w1.T edge-feat rows
    b1 = dp("b1", [P, 2], F32)                            # b1 as [128, chunk]
    w2T = dp("w2T", [2, P, H // 2], BF16)
    b2 = dp("b2", [H // 2], F32)
    w3T = dp("w3T", [P, 1], BF16)
    b3 = dp("b3", [1], F32)

    # ---- per-core data ----
    idx_kv = dp("idx_kv", [P, NT * (P * M // 16)], I16)  # m-major ctx idx per node tile
    idx_type = dp("idx_type", [P, NL // 16], I16)
    idx_cat = dp("idx_cat", [P, NL // 16], I16)
    idx_uv = dp("idx_uv", [P, 2 * EL // 16], I16)
    logd = dp("logd", [1, NL], F32)
    kp = dp("kp", [NL, M], F32)                           # additive pad mask (0 / NEG)
    efT = dp("efT", [2, EL], BF16)

    out_d = nc.declare_dram_parameter("out", [EL], F32, isOutput=True)

    # ---- internal DRAM ----
    kvloc = nc.dram_tensor("kvloc", [NL, 2 * H], BF16)
    xloc = nc.dram_tensor("xloc", [NL, H], BF16)
    kvall = nc.dram_tensor("kvall", [N, 2 * H], BF16, addr_space="Shared")
    xall = nc.dram_tensor("xall", [N, H], BF16, addr_space="Shared")

    groups = [list(range(NC))]
    Alu = mybir.AluOpType
    Act = mybir.ActivationFunctionType

    with tile.TileContext(nc) as tc, ExitStack() as ctx:
        const = ctx.enter_context(tc.tile_pool(name="const", bufs=1))
        xpool = ctx.enter_context(tc.tile_pool(name="xpool", bufs=1))

        # ---------------- constants into SBUF ----------------
        gather = nc.gpsimd.dma_gather
        reg_nl = nc.gpsimd.to_reg(NL)
        reg_pm = nc.gpsimd.to_reg(P * M)
        reg_e2 = nc.gpsimd.to_reg(EL // 2)

        ident = const.tile([P, P], BF16)
        make_identity(nc, ident)

        def bcast_row(dram_ap, n, name):
            t = const.tile([P, n], F32, tag=name, name=name)
            src = bass.AP(tensor=dram_ap.tensor, offset=dram_ap.offset,
                          ap=[[0, P]] + dram_ap.ap)
            nc.sync.dma_start(out=t[:], in_=src)
            return t

        pb_b = bcast_row(proj_b[:], H, "pb")
        bq_b = [bcast_row(bq[ll, :], H, f"bq{ll}") for ll in range(L)]
        bo_b = [bcast_row(bo[ll, :], H, f"bo{ll}") for ll in range(L)]

        db_sb = const.tile([H // 4, 1], F32)
        nc.sync.dma_start(out=db_sb[:], in_=db.rearrange("(p o) -> p o", o=1))
        dw_sb = const.tile([1, H // 4], F32)
        nc.sync.dma_start(out=dw_sb[:], in_=dw[:])
        b1_sb = const.tile([P, 2], F32)
        nc.sync.dma_start(out=b1_sb[:], in_=b1[:])
        b2_sb = const.tile([H // 2, 1], F32)
        nc.sync.dma_start(out=b2_sb[:], in_=b2.rearrange("(p o) -> p o", o=1))
        b3_sb = const.tile([1, 1], F32)
        nc.sync.dma_start(out=b3_sb[:], in_=b3.rearrange("(p o) -> p o", o=1))

        ikv_sb = const.tile([P, NT * P * M // 16], I16)
        nc.sync.dma_start(out=ikv_sb[:], in_=idx_kv[:])
        ity_sb = const.tile([P, NL // 16], I16)
        nc.sync.dma_start(out=ity_sb[:], in_=idx_type[:])
        ica_sb = const.tile([P, NL // 16], I16)
        nc.sync.dma_start(out=ica_sb[:], in_=idx_cat[:])
        iuv_sb = const.tile([P, 2 * EL // 16], I16)
        nc.sync.dma_start(out=iuv_sb[:], in_=idx_uv[:])

        kp_sb = const.tile([P, NT, M], F32)
        nc.sync.dma_start(out=kp_sb[:], in_=kp.rearrange("(t p) m -> p t m", p=P))
        logd_sb = const.tile([1, NL], F32)
        nc.sync.dma_start(out=logd_sb[:], in_=logd[:])

        pw_sb = const.tile([P, 3, H], BF16)
        nc.sync.dma_start(out=pw_sb[:], in_=projWT.rearrange("c p o -> p c o"))
        w1_sb = const.tile([P, 4, H], BF16)
        nc.sync.dma_start(out=w1_sb[:], in_=w1T.rearrange("c p o -> p c o"))
        w1e_sb = const.tile([2, H], BF16)
        nc.sync.dma_start(out=w1e_sb[:], in_=w1eT[:])
        w2_sb = const.tile([P, 2, H // 2], BF16)
        nc.sync.dma_start(out=w2_sb[:], in_=w2T.rearrange("c p o -> p c o"))
        w3_sb = const.tile([P, 1], BF16)
        nc.sync.dma_start(out=w3_sb[:], in_=w3T[:])

        x_sb = xpool.tile([P, NT, H], BF16)

        # ---------------- node feature encoding (scoped pools) ----------------
        with ExitStack() as ectx:
            enc = ectx.enter_context(tc.tile_pool(name="enc", bufs=1))
            epsum = ectx.enter_context(tc.tile_pool(name="epsum", bufs=2, space="PSUM"))
            teT = enc.tile([P, NL], BF16)
            gather(teT.rearrange("p (c n) -> p c n", c=1), type_tab[:],
                                 ity_sb[:], NL, reg_nl, H // 2, transpose=True, single_packet=False)
            ccT = enc.tile([P, NL], BF16)
            gather(ccT.rearrange("p (c n) -> p c n", c=1), cat_tab[:],
                                 ica_sb[:], NL, reg_nl, 2 * CD, transpose=True, single_packet=False)
            deT = enc.tile([P, NL], BF16)
            nc.vector.memset(deT[:], 0.0)
            for s in range(NL // 512):
                pd = epsum.tile([H // 4, 512], F32, tag="pdeg", name="pd")
                nc.tensor.matmul(pd[:], dw_sb[:], logd_sb[:, s * 512:(s + 1) * 512],
                                 start=True, stop=True)
                nc.scalar.activation(deT[0:H // 4, s * 512:(s + 1) * 512], pd[:],
                                     Act.Relu, bias=db_sb[:])
            for g in range(NT):
                px = epsum.tile([P, H], F32, tag="px", name="px")
                cs = slice(g * P, (g + 1) * P)
                nc.tensor.matmul(px[:], teT[:, cs], pw_sb[:, 0, :], start=True, stop=False)
                nc.tensor.matmul(px[:], ccT[:, cs], pw_sb[:, 1, :], start=False, stop=False)
                nc.tensor.matmul(px[:], deT[:, cs], pw_sb[:, 2, :], start=False, stop=True)
                nc.vector.tensor_tensor(x_sb[:, g, :], px[:], pb_b[:], op=Alu.add)

        work = ctx.enter_context(tc.tile_pool(name="work", bufs=1))
        gath = ctx.enter_context(tc.tile_pool(name="gath", bufs=2))
        att = ctx.enter_context(tc.tile_pool(name="att", bufs=2))
        psum = ctx.enter_context(tc.tile_pool(name="psum", bufs=2, space="PSUM"))
        psum1 = ctx.enter_context(tc.tile_pool(name="psum1", bufs=2, space="PSUM"))

        # ---------------- attention layers ----------------
        for ll in range(L):
            wq_sb = work.tile([P, 2, H], BF16, tag="wq", name="wq")
            wk_sb = work.tile([P, 2, H], BF16, tag="wk", name="wk")
            wv_sb = work.tile([P, 2, H], BF16, tag="wv", name="wv")
            wo_sb = work.tile([P, 2, H], BF16, tag="wo", name="wo")
            nc.sync.dma_start(out=wq_sb[:], in_=wqT[ll].rearrange("c p o -> p c o"))
            nc.sync.dma_start(out=wk_sb[:], in_=wkT[ll].rearrange("c p o -> p c o"))
            nc.sync.dma_start(out=wv_sb[:], in_=wvT[ll].rearrange("c p o -> p c o"))
            nc.sync.dma_start(out=wo_sb[:], in_=woT[ll].rearrange("c p o -> p c o"))

            # x^T tiles (lhsT for projections)
            xT = work.tile([P, 2, NT, P], BF16, tag="xT", name="xT")
            for g in range(NT):
                for c in range(2):
                    pt = psum1.tile([P, P], BF16, tag="ptr", name="pt")
                    nc.tensor.transpose(pt[:], x_sb[:, g, c * P:(c + 1) * P], ident[:])
                    nc.scalar.activation(xT[:, c, g, :], pt[:], Act.Copy)

            q_sb = work.tile([P, NT, H], BF16, tag="q", name="q_sb")
            kvall_pview = kvloc.rearrange("(t p) o -> p t o", p=P)
            for g in range(NT):
                pq = psum.tile([P, H], F32, tag="pmm", name="pq")
                nc.tensor.matmul(pq[:], xT[:, 0, g, :], wq_sb[:, 0, :], start=True, stop=False)
                nc.tensor.matmul(pq[:], xT[:, 1, g, :], wq_sb[:, 1, :], start=False, stop=True)
                nc.vector.tensor_tensor(q_sb[:, g, :], pq[:], bq_b[ll][:], op=Alu.add)
                pk = psum.tile([P, H], F32, tag="pmm", name="pk")
                nc.tensor.matmul(pk[:], xT[:, 0, g, :], wk_sb[:, 0, :], start=True, stop=False)
                nc.tensor.matmul(pk[:], xT[:, 1, g, :], wk_sb[:, 1, :], start=False, stop=True)
                kev = work.tile([P, H], BF16, tag="kev", name="kev", bufs=2)
                nc.scalar.activation(kev[:], pk[:], Act.Copy)
                nc.sync.dma_start(out=kvall_pview[:, g, 0:H], in_=kev[:])
                pv = psum.tile([P, H], F32, tag="pmm", name="pv")
                nc.tensor.matmul(pv[:], xT[:, 0, g, :], wv_sb[:, 0, :], start=True, stop=False)
                nc.tensor.matmul(pv[:], xT[:, 1, g, :], wv_sb[:, 1, :], start=False, stop=True)
                vev = work.tile([P, H], BF16, tag="vev", name="vev", bufs=2)
                nc.scalar.activation(vev[:], pv[:], Act.Copy)
                nc.sync.dma_start(out=kvall_pview[:, g, H:2 * H], in_=vev[:])

            nc.gpsimd.collective_compute("AllGather", Alu.bypass, replica_groups=groups,
                                         ins=[kvloc[:]], outs=[kvall[:]])

            for t in range(NT):
                isl = ikv_sb[:, t * (P * M // 16):(t + 1) * (P * M // 16)]
                kvg = gath.tile([P, M, 2 * H], BF16, tag="kg", name="kvg")
                gather(kvg[:], kvall[:], isl, P * M, reg_pm, 2 * H, single_packet=False)
                kg = kvg[:, :, 0:H]
                vg = kvg[:, :, H:2 * H]

                # scores: s[n,m,h] = sum_d q*k  (d-tree reduce, ping-pong pp<->ta)
                pp = att.tile([P, M, HEADS, DH], BF16, tag="pp", name="pp")
                qb = q_sb[:, t, None, :].to_broadcast([P, M, H])
                nc.vector.tensor_tensor(pp.rearrange("p m h d -> p m (h d)"),
                                        kg.rearrange("p m o -> p m o"), qb, op=Alu.mult)
                ta = att.tile([P, M, HEADS, DH // 2], BF16, tag="ta", name="ta")
                nc.vector.tensor_tensor(ta[:], pp[:, :, :, 0:32], pp[:, :, :, 32:64], op=Alu.add)
                nc.vector.tensor_tensor(pp[:, :, :, 0:16], ta[:, :, :, 0:16], ta[:, :, :, 16:32], op=Alu.add)
                nc.vector.tensor_tensor(ta[:, :, :, 0:8], pp[:, :, :, 0:8], pp[:, :, :, 8:16], op=Alu.add)
                nc.vector.tensor_tensor(pp[:, :, :, 0:4], ta[:, :, :, 0:4], ta[:, :, :, 4:8], op=Alu.add)
                nc.vector.tensor_tensor(ta[:, :, :, 0:2], pp[:, :, :, 0:2], pp[:, :, :, 2:4], op=Alu.add)
                s_m = att.tile([P, M, HEADS], F32, tag="sm", name="s_m")
                nc.vector.tensor_tensor(s_m[:], ta[:, :, :, 0], ta[:, :, :, 1], op=Alu.add)

                kpb = kp_sb[:, t, :, None].to_broadcast([P, M, HEADS])
                nc.vector.tensor_tensor(s_m[:], s_m[:], kpb, op=Alu.add)
                es = att.tile([P, M, HEADS], F32, tag="es", name="es")
                nc.scalar.activation(es[:], s_m[:], Act.Exp)
                sums = att.tile([P, HEADS], F32, tag="sums", name="sums")
                nc.vector.tensor_reduce(sums[:], es.rearrange("p m h -> p h m"),
                                        axis=mybir.AxisListType.X, op=Alu.add)
                rs = att.tile([P, HEADS], F32, tag="rs", name="rs")
                nc.vector.reciprocal(rs[:], sums[:])
                attw = att.tile([P, M, HEADS], BF16, tag="attw", name="attw")
                nc.vector.tensor_tensor(attw[:], es[:],
                                        rs[:, None, :].to_broadcast([P, M, HEADS]), op=Alu.mult)

                # AV: o[n,:] = sum_m attw * v  (m-tree, ping-pong av<->tm)
                av = att.tile([P, M, H], BF16, tag="pp", name="av")
                nc.vector.tensor_tensor(av.rearrange("p m (h d) -> p m h d", h=HEADS),
                                        vg.rearrange("p m (h d) -> p m h d", h=HEADS),
                                        attw[:, :, :, None].to_broadcast([P, M, HEADS, DH]),
                                        op=Alu.mult)
                tm = att.tile([P, M // 2, H], BF16, tag="ta", name="tm")
                nc.vector.tensor_tensor(tm[:], av[:, 0:16, :], av[:, 16:32, :], op=Alu.add)
                nc.vector.tensor_tensor(av[:, 0:8, :], tm[:, 0:8, :], tm[:, 8:16, :], op=Alu.add)
                nc.vector.tensor_tensor(tm[:, 0:4, :], av[:, 0:4, :], av[:, 4:8, :], op=Alu.add)
                nc.vector.tensor_tensor(av[:, 0:2, :], tm[:, 0:2, :], tm[:, 2:4, :], op=Alu.add)
                o_sb = att.tile([P, H], BF16, tag="o", name="o_sb")
                nc.vector.tensor_tensor(o_sb[:], av[:, 0, :], av[:, 1, :], op=Alu.add)

                # out-proj + relu -> x
                oT = att.tile([P, 2, P], BF16, tag="oT", name="oT")
                for c in range(2):
                    pt = psum1.tile([P, P], BF16, tag="ptr", name="pt")
                    nc.tensor.transpose(pt[:], o_sb[:, c * P:(c + 1) * P], ident[:])
                    nc.scalar.activation(oT[:, c, :], pt[:], Act.Copy)
                pxn = psum.tile([P, H], F32, tag="pmm", name="pxn")
                nc.tensor.matmul(pxn[:], oT[:, 0, :], wo_sb[:, 0, :], start=True, stop=False)
                nc.tensor.matmul(pxn[:], oT[:, 1, :], wo_sb[:, 1, :], start=False, stop=True)
                nc.vector.tensor_tensor(x_sb[:, t, :], pxn[:], bo_b[ll][:], op=Alu.add)
                nc.vector.tensor_scalar_max(x_sb[:, t, :], x_sb[:, t, :], 0.0)

        # ---------------- edge MLP ----------------
        nc.sync.dma_start(out=xloc.rearrange("(t p) o -> p t o", p=P), in_=x_sb[:])
        nc.gpsimd.collective_compute("AllGather", Alu.bypass, replica_groups=groups,
                                     ins=[xloc[:]], outs=[xall[:]])

        EQ = EL // 4
        for quar in range(4):
            hsl = slice(quar * (2 * EQ // 16), (quar + 1) * (2 * EQ // 16))
            uvg = gath.tile([P, 2, 2 * EQ], BF16, tag="kg", name="uvg")
            gather(uvg[:], xall[:], iuv_sb[:, hsl], 2 * EQ, reg_e2, H,
                                 transpose=True, single_packet=False)
            ug = uvg[:, :, 0:EQ]
            vg2 = uvg[:, :, EQ:2 * EQ]
            for e in range(EQ // 512):
                eg = quar * (EQ // 512) + e
                esl = slice(e * 512, (e + 1) * 512)
                ef_sb = att.tile([2, 512], BF16, tag="ef", name="ef_sb")
                nc.sync.dma_start(out=ef_sb[:], in_=efT[:, eg * 512:(eg + 1) * 512])
                h1T = att.tile([P, 2, 512], BF16, tag="h1T", name="h1T")
                for oc in range(2):
                    ph = psum.tile([P, 512], F32, tag="pbig", name="ph")
                    ocs = slice(oc * P, (oc + 1) * P)
                    nc.tensor.matmul(ph[:], w1_sb[:, 0, ocs], ug[:, 0, esl], start=True, stop=False)
                    nc.tensor.matmul(ph[:], w1_sb[:, 1, ocs], ug[:, 1, esl], start=False, stop=False)
                    nc.tensor.matmul(ph[:], w1_sb[:, 2, ocs], vg2[:, 0, esl], start=False, stop=False)
                    nc.tensor.matmul(ph[:], w1_sb[:, 3, ocs], vg2[:, 1, esl], start=False, stop=False)
                    nc.tensor.matmul(ph[:], w1e_sb[:, ocs], ef_sb[:], start=False, stop=True)
                    nc.scalar.activation(h1T[:, oc, :], ph[:], Act.Relu, bias=b1_sb[:, oc:oc + 1])
                ph2 = psum.tile([P, 512], F32, tag="pbig", name="ph2")
                nc.tensor.matmul(ph2[0:H // 2, :], w2_sb[:, 0, :], h1T[:, 0, :], start=True, stop=False)
                nc.tensor.matmul(ph2[0:H // 2, :], w2_sb[:, 1, :], h1T[:, 1, :], start=False, stop=True)
                h2T = att.tile([H // 2, 512], BF16, tag="h2T", name="h2T")
                nc.scalar.activation(h2T[:], ph2[0:H // 2, :], Act.Relu, bias=b2_sb[:])
                pl = psum1.tile([1, 512], F32, tag="pl", name="pl")
                nc.tensor.matmul(pl[:], w3_sb[:, :], h2T[:], start=True, stop=True)
                lo = att.tile([1, 512], F32, tag="lo", name="lo")
                nc.scalar.activation(lo[:], pl[:], Act.Identity, bias=b3_sb[:])
                nc.sync.dma_start(out=out_d.rearrange("(a b) -> a b", a=ET)[eg, None, :], in_=lo[:])

    nc.finalize()
    return nc


# --------------------------------------------------------------------------
# Host-side prep + runner
# --------------------------------------------------------------------------

_CACHE = {}


def _prep_maps(inputs):
    f = {k: np.asarray(v) for k, v in inputs.items()}
    scale = 1.0 / np.sqrt(np.float32(DH))

    cat0, cat1 = f["cat_embed0"].astype(np.float32), f["cat_embed1"].astype(np.float32)
    cat_tab = np.zeros((V * V, 2 * CD), np.float32)
    for i0 in range(V):
        for i1 in range(V):
            cat_tab[i0 * V + i1] = np.concatenate([cat0[i0], cat1[i1]])

    projWT = np.zeros((3, P, H), np.float32)
    pwt = f["proj_w"].astype(np.float32).T  # [320, 256]
    projWT[0] = pwt[0:128]
    projWT[1] = pwt[128:256]
    projWT[2, 0:64] = pwt[256:320]

    wqT = np.empty((L, 2, P, H), np.float32)
    wkT = np.empty((L, 2, P, H), np.float32)
    wvT = np.empty((L, 2, P, H), np.float32)
    woT = np.empty((L, 2, P, H), np.float32)
    bq = np.empty((L, H), np.float32)
    bo = np.empty((L, H), np.float32)
    for ll in range(L):
        w = f["in_proj_w"][ll].astype(np.float32)
        b = f["in_proj_b"][ll].astype(np.float32)
        wq, wk, wv = w[0:H], w[H:2 * H], w[2 * H:3 * H]
        bq[ll] = b[0:H] * scale
        bv = b[2 * H:3 * H]
        for c in range(2):
            wqT[ll, c] = (wq * scale).T[c * P:(c + 1) * P]
            wkT[ll, c] = wk.T[c * P:(c + 1) * P]
            wvT[ll, c] = wv.T[c * P:(c + 1) * P]
            woT[ll, c] = f["out_w"][ll].astype(np.float32).T[c * P:(c + 1) * P]
        bo[ll] = f["out_b"][ll].astype(np.float32) + f["out_w"][ll].astype(np.float32) @ bv

    w1 = f["mlp_w1"].astype(np.float32)      # [256, 514]
    w1T_full = w1.T                           # [514, 256]
    w1T = np.stack([w1T_full[c * P:(c + 1) * P] for c in range(4)])
    w1eT = w1T_full[512:514]
    b1 = f["mlp_b1"].astype(np.float32).reshape(2, P).T  # [128, 2]
    w2T = np.stack([f["mlp_w2"].astype(np.float32).T[c * P:(c + 1) * P] for c in range(2)])
    w3T = f["mlp_w3"].astype(np.float32).T   # [128, 1]

    shared = {
        "type_tab": _bf(f["type_embed"]),
        "cat_tab": _bf(cat_tab),
        "dw": _f32(f["degree_w"].reshape(1, -1)),
        "db": _f32(f["degree_b"]),
        "projWT": _bf(projWT),
        "proj_b": _f32(f["proj_b"]),
        "wqT": _bf(wqT), "bq": _f32(bq),
        "wkT": _bf(wkT), "wvT": _bf(wvT),
        "woT": _bf(woT), "bo": _f32(bo),
        "w1T": _bf(w1T), "w1eT": _bf(w1eT), "b1": _f32(b1),
        "w2T": _bf(w2T), "b2": _f32(f["mlp_b2"]),
        "w3T": _bf(w3T), "b3": _f32(f["mlp_b3"]),
    }

    ctx = f["context_indices"].astype(np.int64)
    kpm = f["key_padding_mask"].astype(bool)
    maps = []
    for c in range(NC):
        ns = slice(c * NL, (c + 1) * NL)
        es = slice(c * EL, (c + 1) * EL)
        ctx_c = ctx[ns]  # [2048, 32]
        idx_kv = np.concatenate(
            [ctx_c[t * P:(t + 1) * P].T.flatten() for t in range(NT)])
        m = dict(shared)
        m["idx_kv"] = _wrap16(idx_kv)
        m["idx_type"] = _wrap16(f["type_idx"][ns])
        m["idx_cat"] = _wrap16(f["cat_idx"][ns, 0] * V + f["cat_idx"][ns, 1])
        u_c, v_c = f["u_idx"][es], f["v_idx"][es]
        EQ = EL // 4
        m["idx_uv"] = _wrap16(np.concatenate(
            [np.concatenate([u_c[q * EQ:(q + 1) * EQ], v_c[q * EQ:(q + 1) * EQ]])
             for q in range(4)]))
        m["logd"] = _f32(f["log_degree"][ns].reshape(1, NL))
        m["kp"] = _f32(np.where(kpm[ns], NEG, 0.0))
        m["efT"] = _bf(f["edge_feats"][es].T)
        maps.append(m)
    return maps


def kernel(**inputs):
    if "nc" not in _CACHE:
        _CACHE["nc"] = build_program()
    nc = _CACHE["nc"]
    maps = _prep_maps(inputs)
    res = run_bass_kernel_spmd(nc, maps, core_ids=list(range(NC)))
    return np.concatenate([res.results[c]["out"] for c in range(NC)]).astype(np.float32)


if __name__ == "__main__":
    nc = build_program()
    print("program built OK")

